# revision 34
# baseline (speedup 1.0000x reference)
"""L1-attention kernel for Trainium2 (8 NeuronCores).

attn[b, i, j, h] = -(1/sqrt(W)) * sum_w |q[b,j,h,w] - k[b,i,h,w]|

Strategy (thermometer/sign-code dense matmul):
  Shard (batch x head-pair) across the 8 cores. Quantize each input
  element to a uniform grid of T=20 thresholds over [-3, 3] and encode
  it as a sign vector c_t(x) = (1[x > tau_t] - 1/2). For two such
  codes, dot(c(a), c(b)) = (1/4)(K - 2*sum_t XOR_t) with
  sum_t XOR_t = |L(a) - L(b)| (threshold-crossing count), so

      sum_w |a_w - b_w| ~= delta * (32*T - 2*dot(Cq, Ck))

  i.e. the ENTIRE pairwise L1 reduction becomes one dense fp8 matmul
  with contraction dim 64*T = 1280 per head, run on the PE in
  DoubleRow mode (256-row contraction per instruction, ~216 ns per
  [256 x 128 x 512] matmul warm). The +-1/2 codes are exact in fp8
  and self-correcting (no Sq/Sk bias terms), so the device does only
  matmuls plus a fused scale/bias DVE evacuation to bf16.

  Schedule: codes are host-encoded and streamed on the sync HWDGE
  ring in consumption order ([4,6]-chunk slabs per head/side,
  ~300 GB/s); output tiles leave on the scalar and sync rings
  alternately. Ten full-width warm-up matmuls on a zero tile run
  during the DMA fill so the PE HAM clock-gate releases (1.2 ->
  2.4 GHz) before the real matmuls arrive. Rel err ~1.38e-2
  (quantization-dominated), HW exec ~27 us.
"""

import sys

sys.path.insert(0, "/opt/trn_rl_repo")

import numpy as np

BS, N_CTX, N_HEADS, WIDTH = 2, 512, 8, 64
N_CORES = 8

T = 20  # thermometer levels
R = 3.0  # clip range
DELTA = 2.0 * R / T
NCC = T * WIDTH // 128  # 128-row contraction chunks per head
NCP = NCC // 2  # DoubleRow chunk-pairs
SCALE_MM = DELTA / 4.0
BIAS_MM = -4.0 * T * DELTA
N_WARM = 8  # PE HAM warm-up matmuls (full-width)
SLABS = [(0, 4), (4, 10)]  # input DMA slab boundaries (chunks)

_CACHE = {}


def _build():
    if "nc" in _CACHE:
        return _CACHE["nc"]

    import concourse.bacc as bacc
    import concourse.mybir as mybir
    import concourse.tile as tile

    fp8 = mybir.dt.float8e4
    fp32 = mybir.dt.float32
    bf16 = mybir.dt.bfloat16

    nc = bacc.Bacc(
        "TRN2",
        target_bir_lowering=False,
        debug=False,
        enable_asserts=False,
        num_devices=N_CORES,
    )

    aq_d = nc.dram_tensor("aq", [128, 2, NCC, N_CTX], fp8, kind="ExternalInput")
    ak_d = nc.dram_tensor("ak", [128, 2, NCC, N_CTX], fp8, kind="ExternalInput")
    out_d = nc.dram_tensor("out", [2, N_CTX, N_CTX], bf16, kind="ExternalOutput")

    with tile.TileContext(nc) as tc:
        with (
            tc.tile_pool(name="codes", bufs=1) as cp,
            tc.tile_pool(name="ps", bufs=8, space="PSUM") as pp,
            tc.tile_pool(name="o", bufs=4) as op,
        ):
            aq = cp.tile([128, 2, NCC, N_CTX], fp8)
            ak = cp.tile([128, 2, NCC, N_CTX], fp8)
            warm = cp.tile([128, N_CTX], fp8)
            nc.gpsimd.memset(warm[:], 0)
            biasc = cp.tile([128, 1], fp32)
            nc.gpsimd.memset(biasc[:], BIAS_MM)
            # dummy activation: pull the ACT table load into the fill phase
            actw = cp.tile([128, 1], fp32)
            nc.scalar.activation(
                actw[:], biasc[:], mybir.ActivationFunctionType.Identity
            )

            # h0 inputs interleaved on the sync HWDGE ring (in
            # consumption order); h1 inputs stream concurrently on the
            # two SWDGE queues (vector/gpsimd) so all three move in
            # parallel. Outputs later use the scalar HWDGE ring.
            for h in range(2):
                for a, b in SLABS:
                    # lead ak slab rides the otherwise-idle scalar ring so
                    # both lead slabs land in parallel and the PE starts
                    # ~1.3us earlier; everything else streams on sync
                    eng = nc.scalar if (h == 0 and a == 0) else nc.sync
                    eng.dma_start(ak[:, h, a:b, :], ak_d[:, h, a:b, :])
                    nc.sync.dma_start(aq[:, h, a:b, :], aq_d[:, h, a:b, :])

            # HAM warm-up: keep the PE busy from t~0 so it un-throttles
            # to 2.4 GHz before the real matmuls arrive.
            wps = pp.tile([128, N_CTX], fp32, tag="ps", name="wps")
            for i in range(N_WARM):
                nc.tensor.matmul(
                    wps[:],
                    warm[:, 0:128],
                    warm[:],
                    start=True,
                    stop=True,
                )

            for h in range(2):
                ps = [
                    pp.tile([128, N_CTX], fp32, tag="ps", name=f"ps_{h}_{kc}")
                    for kc in range(4)
                ]
                for ccp in range(NCP):
                    s = slice(2 * ccp, 2 * ccp + 2)
                    for kc in range(4):
                        nc.tensor.matmul(
                            ps[kc][:],
                            ak[:, h, s, kc * 128 : (kc + 1) * 128],
                            aq[:, h, s, :],
                            start=(ccp == 0),
                            stop=(ccp == NCP - 1),
                            perf_mode=mybir.MatmulPerfMode.DoubleRow,
                        )
                for kc in range(4):
                    ot = op.tile([128, N_CTX], bf16, tag="o", name=f"o_{h}_{kc}")
                    # last head: halve the final two tiles so the last
                    # output DMA is small and lands early; odd tiles
                    # evacuate on ACT so DVE and ACT drain in parallel
                    nsub = 2 if (h == 1 and kc >= 2) else 1
                    w = N_CTX // nsub
                    for sub in range(nsub):
                        cs = slice(sub * w, (sub + 1) * w)
                        if h == 1 and kc % 2 == 1:
                            nc.scalar.activation(
                                ot[:, cs],
                                ps[kc][:, cs],
                                mybir.ActivationFunctionType.Identity,
                                bias=biasc[:, 0:1],
                                scale=SCALE_MM,
                            )
                        else:
                            nc.vector.tensor_scalar(
                                ot[:, cs],
                                ps[kc][:, cs],
                                SCALE_MM,
                                BIAS_MM,
                                mybir.AluOpType.mult,
                                mybir.AluOpType.add,
                            )
                        oeng = nc.scalar if kc % 2 == 0 else nc.sync
                        oeng.dma_start(
                            out_d[h, kc * 128 : (kc + 1) * 128, cs], ot[:, cs]
                        )

    nc.compile()
    _CACHE["nc"] = nc
    return nc


def _encode(x):
    """x: [BS, N_CTX, N_HEADS, WIDTH] -> codes [BS, N_HEADS, 128, NCC, N_CTX] fp8."""
    import concourse.mybir as mybir

    fp8np = mybir.dt.np(mybir.dt.float8e4)
    taus = (-R + DELTA * (np.arange(T) + 0.5)).astype(np.float32)
    xt = x.transpose(0, 2, 3, 1)  # [b, h, w, j]
    bits = xt[:, :, None, :, :] > taus[None, None, :, None, None]  # [b,h,T,w,j]
    codes = np.where(bits, np.float32(0.5), np.float32(-0.5))
    # contraction row r = t*W + w; chunk cc = r // 128, partition p = r % 128
    codes = codes.reshape(BS, N_HEADS, NCC, 128, N_CTX).transpose(0, 1, 3, 2, 4)
    return np.ascontiguousarray(codes.astype(fp8np))


def kernel(q, k, _trace=False):
    from concourse.bass_utils import run_bass_kernel_spmd

    q = np.asarray(q, dtype=np.float32)
    k = np.asarray(k, dtype=np.float32)
    nc = _build()
    cq = _encode(q)  # [b, h, 128, NCC, j]
    ck = _encode(k)
    in_maps = []
    for c in range(N_CORES):
        b, hp = divmod(c, 4)
        aq = np.ascontiguousarray(
            cq[b, 2 * hp : 2 * hp + 2].transpose(1, 0, 2, 3)
        )  # [128, 2, NCC, 512]
        ak = np.ascontiguousarray(ck[b, 2 * hp : 2 * hp + 2].transpose(1, 0, 2, 3))
        in_maps.append({"aq": aq, "ak": ak})
    res = run_bass_kernel_spmd(nc, in_maps, core_ids=list(range(N_CORES)), trace=_trace)
    _CACHE["last_results"] = res
    attn = np.empty((BS, N_CTX, N_CTX, N_HEADS), np.float32)
    for c in range(N_CORES):
        b, hp = divmod(c, 4)
        o = res.results[c]["out"].astype(np.float32)
        attn[b, :, :, 2 * hp] = o[0]
        attn[b, :, :, 2 * hp + 1] = o[1]
    return attn


# revision 35
# speedup vs baseline: 1.0681x; 1.0681x over previous
"""L1-attention kernel for Trainium2 (8 NeuronCores).

attn[b, i, j, h] = -(1/sqrt(W)) * sum_w |q[b,j,h,w] - k[b,i,h,w]|

Strategy (thermometer/sign-code dense matmul):
  Shard (batch x head-pair) across the 8 cores. Quantize each input
  element to a uniform grid of T=20 thresholds over [-3, 3] and encode
  it as a sign vector c_t(x) = (1[x > tau_t] - 1/2). For two such
  codes, dot(c(a), c(b)) = (1/4)(K - 2*sum_t XOR_t) with
  sum_t XOR_t = |L(a) - L(b)| (threshold-crossing count), so

      sum_w |a_w - b_w| ~= delta * (32*T - 2*dot(Cq, Ck))

  i.e. the ENTIRE pairwise L1 reduction becomes one dense fp8 matmul
  with contraction dim 64*T = 1280 per head, run on the PE in
  DoubleRow mode (256-row contraction per instruction, ~216 ns per
  [256 x 128 x 512] matmul warm). The +-1/2 codes are exact in fp8
  and self-correcting (no Sq/Sk bias terms), so the device does only
  matmuls plus a fused scale/bias DVE evacuation to bf16.

  Schedule: codes are host-encoded and streamed on the sync HWDGE
  ring in consumption order ([4,6]-chunk slabs per head/side,
  ~300 GB/s); output tiles leave on the scalar and sync rings
  alternately. Ten full-width warm-up matmuls on a zero tile run
  during the DMA fill so the PE HAM clock-gate releases (1.2 ->
  2.4 GHz) before the real matmuls arrive. Rel err ~1.38e-2
  (quantization-dominated), HW exec ~27 us.
"""

import sys

sys.path.insert(0, "/opt/trn_rl_repo")

import numpy as np

BS, N_CTX, N_HEADS, WIDTH = 2, 512, 8, 64
N_CORES = 8

T = 20  # thermometer levels
R = 3.0  # clip range
DELTA = 2.0 * R / T
NCC = T * WIDTH // 128  # 128-row contraction chunks per head
NCP = NCC // 2  # DoubleRow chunk-pairs
SCALE_MM = DELTA / 4.0
BIAS_MM = -4.0 * T * DELTA
N_WARM = 10  # PE HAM warm-up matmuls (full-width)
SLABS = [(0, 4), (4, 10)]  # input DMA slab boundaries (chunks)

_CACHE = {}


def _build():
    if "nc" in _CACHE:
        return _CACHE["nc"]

    import concourse.bacc as bacc
    import concourse.mybir as mybir
    import concourse.tile as tile

    fp8 = mybir.dt.float8e4
    fp32 = mybir.dt.float32
    bf16 = mybir.dt.bfloat16

    nc = bacc.Bacc(
        "TRN2",
        target_bir_lowering=False,
        debug=False,
        enable_asserts=False,
        num_devices=N_CORES,
    )

    aq_d = nc.dram_tensor("aq", [128, 2, NCC, N_CTX], fp8, kind="ExternalInput")
    ak_d = nc.dram_tensor("ak", [128, 2, NCC, N_CTX], fp8, kind="ExternalInput")
    out_d = nc.dram_tensor("out", [2, N_CTX, N_CTX], bf16, kind="ExternalOutput")

    with tile.TileContext(nc) as tc:
        with (
            tc.tile_pool(name="codes", bufs=1) as cp,
            tc.tile_pool(name="ps", bufs=8, space="PSUM") as pp,
            tc.tile_pool(name="o", bufs=4) as op,
        ):
            aq = cp.tile([128, 2, NCC, N_CTX], fp8)
            ak = cp.tile([128, 2, NCC, N_CTX], fp8)
            warm = cp.tile([128, N_CTX], fp8)
            nc.gpsimd.memset(warm[:], 0)
            biasc = cp.tile([128, 1], fp32)
            nc.gpsimd.memset(biasc[:], BIAS_MM)
            # dummy activation: pull the ACT table load into the fill phase
            actw = cp.tile([128, 1], fp32)
            nc.scalar.activation(
                actw[:], biasc[:], mybir.ActivationFunctionType.Identity
            )

            # h0 inputs interleaved on the sync HWDGE ring (in
            # consumption order); h1 inputs stream concurrently on the
            # two SWDGE queues (vector/gpsimd) so all three move in
            # parallel. Outputs later use the scalar HWDGE ring.
            for h in range(2):
                for a, b in SLABS:
                    # lead ak slab rides the otherwise-idle scalar ring so
                    # both lead slabs land in parallel and the PE starts
                    # ~1.3us earlier; everything else streams on sync
                    eng = nc.scalar if (h == 0 and a == 0) else nc.sync
                    eng.dma_start(ak[:, h, a:b, :], ak_d[:, h, a:b, :])
                    nc.sync.dma_start(aq[:, h, a:b, :], aq_d[:, h, a:b, :])

            # HAM warm-up: keep the PE busy from t~0 so it un-throttles
            # to 2.4 GHz before the real matmuls arrive.
            wps = pp.tile([128, N_CTX], fp32, tag="ps", name="wps")
            for i in range(N_WARM):
                nc.tensor.matmul(
                    wps[:],
                    warm[:, 0:128],
                    warm[:],
                    start=True,
                    stop=True,
                )

            for h in range(2):
                ps = [
                    pp.tile([128, N_CTX], fp32, tag="ps", name=f"ps_{h}_{kc}")
                    for kc in range(4)
                ]
                for ccp in range(NCP):
                    s = slice(2 * ccp, 2 * ccp + 2)
                    for kc in range(4):
                        nc.tensor.matmul(
                            ps[kc][:],
                            ak[:, h, s, kc * 128 : (kc + 1) * 128],
                            aq[:, h, s, :],
                            start=(ccp == 0),
                            stop=(ccp == NCP - 1),
                            perf_mode=mybir.MatmulPerfMode.DoubleRow,
                        )
                for kc in range(4):
                    ot = op.tile([128, N_CTX], bf16, tag="o", name=f"o_{h}_{kc}")
                    # odd tiles of the last head evacuate on ACT so the
                    # DVE and ACT evacuations drain the tail in parallel
                    nsub = 1
                    w = N_CTX // nsub
                    for sub in range(nsub):
                        cs = slice(sub * w, (sub + 1) * w)
                        if h == 1 and kc % 2 == 1:
                            nc.scalar.activation(
                                ot[:, cs],
                                ps[kc][:, cs],
                                mybir.ActivationFunctionType.Identity,
                                bias=biasc[:, 0:1],
                                scale=SCALE_MM,
                            )
                        else:
                            nc.vector.tensor_scalar(
                                ot[:, cs],
                                ps[kc][:, cs],
                                SCALE_MM,
                                BIAS_MM,
                                mybir.AluOpType.mult,
                                mybir.AluOpType.add,
                            )
                        oeng = nc.scalar if kc % 2 == 0 else nc.sync
                        oeng.dma_start(
                            out_d[h, kc * 128 : (kc + 1) * 128, cs], ot[:, cs]
                        )

    nc.compile()
    _CACHE["nc"] = nc
    return nc


def _encode(x):
    """x: [BS, N_CTX, N_HEADS, WIDTH] -> codes [BS, N_HEADS, 128, NCC, N_CTX] fp8."""
    import concourse.mybir as mybir

    fp8np = mybir.dt.np(mybir.dt.float8e4)
    taus = (-R + DELTA * (np.arange(T) + 0.5)).astype(np.float32)
    xt = x.transpose(0, 2, 3, 1)  # [b, h, w, j]
    bits = xt[:, :, None, :, :] > taus[None, None, :, None, None]  # [b,h,T,w,j]
    codes = np.where(bits, np.float32(0.5), np.float32(-0.5))
    # contraction row r = t*W + w; chunk cc = r // 128, partition p = r % 128
    codes = codes.reshape(BS, N_HEADS, NCC, 128, N_CTX).transpose(0, 1, 3, 2, 4)
    return np.ascontiguousarray(codes.astype(fp8np))


def kernel(q, k, _trace=False):
    from concourse.bass_utils import run_bass_kernel_spmd

    q = np.asarray(q, dtype=np.float32)
    k = np.asarray(k, dtype=np.float32)
    nc = _build()
    cq = _encode(q)  # [b, h, 128, NCC, j]
    ck = _encode(k)
    in_maps = []
    for c in range(N_CORES):
        b, hp = divmod(c, 4)
        aq = np.ascontiguousarray(
            cq[b, 2 * hp : 2 * hp + 2].transpose(1, 0, 2, 3)
        )  # [128, 2, NCC, 512]
        ak = np.ascontiguousarray(ck[b, 2 * hp : 2 * hp + 2].transpose(1, 0, 2, 3))
        in_maps.append({"aq": aq, "ak": ak})
    res = run_bass_kernel_spmd(nc, in_maps, core_ids=list(range(N_CORES)), trace=_trace)
    _CACHE["last_results"] = res
    attn = np.empty((BS, N_CTX, N_CTX, N_HEADS), np.float32)
    for c in range(N_CORES):
        b, hp = divmod(c, 4)
        o = res.results[c]["out"].astype(np.float32)
        attn[b, :, :, 2 * hp] = o[0]
        attn[b, :, :, 2 * hp + 1] = o[1]
    return attn


# revision 36
# speedup vs baseline: 1.0843x; 1.0152x over previous
"""L1-attention kernel for Trainium2 (8 NeuronCores).

attn[b, i, j, h] = -(1/sqrt(W)) * sum_w |q[b,j,h,w] - k[b,i,h,w]|

Strategy (thermometer/sign-code dense matmul):
  Shard (batch x head-pair) across the 8 cores. Quantize each input
  element to a uniform grid of T=20 thresholds over [-3, 3] and encode
  it as a sign vector c_t(x) = (1[x > tau_t] - 1/2). For two such
  codes, dot(c(a), c(b)) = (1/4)(K - 2*sum_t XOR_t) with
  sum_t XOR_t = |L(a) - L(b)| (threshold-crossing count), so

      sum_w |a_w - b_w| ~= delta * (32*T - 2*dot(Cq, Ck))

  i.e. the ENTIRE pairwise L1 reduction becomes one dense fp8 matmul
  with contraction dim 64*T = 1280 per head, run on the PE in
  DoubleRow mode (256-row contraction per instruction, ~216 ns per
  [256 x 128 x 512] matmul warm). The +-1/2 codes are exact in fp8
  and self-correcting (no Sq/Sk bias terms), so the device does only
  matmuls plus a fused scale/bias DVE evacuation to bf16.

  Schedule: codes are host-encoded and streamed on the sync HWDGE
  ring in consumption order ([4,6]-chunk slabs per head/side,
  ~300 GB/s); output tiles leave on the scalar and sync rings
  alternately. Ten full-width warm-up matmuls on a zero tile run
  during the DMA fill so the PE HAM clock-gate releases (1.2 ->
  2.4 GHz) before the real matmuls arrive. Rel err ~1.38e-2
  (quantization-dominated), HW exec ~27 us.
"""

import sys

sys.path.insert(0, "/opt/trn_rl_repo")

import numpy as np

BS, N_CTX, N_HEADS, WIDTH = 2, 512, 8, 64
N_CORES = 8

T = 20  # thermometer levels
R = 3.0  # clip range
DELTA = 2.0 * R / T
NCC = T * WIDTH // 128  # 128-row contraction chunks per head
NCP = NCC // 2  # DoubleRow chunk-pairs
SCALE_MM = DELTA / 4.0
BIAS_MM = -4.0 * T * DELTA
N_WARM = 10  # PE HAM warm-up matmuls (full-width)
SLABS = [(0, 6), (6, 10)]  # input DMA slab boundaries (chunks)

_CACHE = {}


def _build():
    if "nc" in _CACHE:
        return _CACHE["nc"]

    import concourse.bacc as bacc
    import concourse.mybir as mybir
    import concourse.tile as tile

    fp8 = mybir.dt.float8e4
    fp32 = mybir.dt.float32
    bf16 = mybir.dt.bfloat16

    nc = bacc.Bacc(
        "TRN2",
        target_bir_lowering=False,
        debug=False,
        enable_asserts=False,
        num_devices=N_CORES,
    )

    aq_d = nc.dram_tensor("aq", [128, 2, NCC, N_CTX], fp8, kind="ExternalInput")
    ak_d = nc.dram_tensor("ak", [128, 2, NCC, N_CTX], fp8, kind="ExternalInput")
    out_d = nc.dram_tensor("out", [2, N_CTX, N_CTX], bf16, kind="ExternalOutput")

    with tile.TileContext(nc) as tc:
        with (
            tc.tile_pool(name="codes", bufs=1) as cp,
            tc.tile_pool(name="ps", bufs=8, space="PSUM") as pp,
            tc.tile_pool(name="o", bufs=4) as op,
        ):
            aq = cp.tile([128, 2, NCC, N_CTX], fp8)
            ak = cp.tile([128, 2, NCC, N_CTX], fp8)
            warm = cp.tile([128, N_CTX], fp8)
            nc.gpsimd.memset(warm[:], 0)
            biasc = cp.tile([128, 1], fp32)
            nc.gpsimd.memset(biasc[:], BIAS_MM)
            # dummy activation: pull the ACT table load into the fill phase
            actw = cp.tile([128, 1], fp32)
            nc.scalar.activation(
                actw[:], biasc[:], mybir.ActivationFunctionType.Identity
            )

            # h0 inputs interleaved on the sync HWDGE ring (in
            # consumption order); h1 inputs stream concurrently on the
            # two SWDGE queues (vector/gpsimd) so all three move in
            # parallel. Outputs later use the scalar HWDGE ring.
            for h in range(2):
                for a, b in SLABS:
                    # lead ak slab rides the otherwise-idle scalar ring so
                    # both lead slabs land in parallel and the PE starts
                    # ~1.3us earlier; everything else streams on sync
                    eng = nc.scalar if (h == 0 and a == 0) else nc.sync
                    eng.dma_start(ak[:, h, a:b, :], ak_d[:, h, a:b, :])
                    nc.sync.dma_start(aq[:, h, a:b, :], aq_d[:, h, a:b, :])

            # HAM warm-up: keep the PE busy from t~0 so it un-throttles
            # to 2.4 GHz before the real matmuls arrive.
            wps = pp.tile([128, N_CTX], fp32, tag="ps", name="wps")
            for i in range(N_WARM):
                nc.tensor.matmul(
                    wps[:],
                    warm[:, 0:128],
                    warm[:],
                    start=True,
                    stop=True,
                )

            for h in range(2):
                ps = [
                    pp.tile([128, N_CTX], fp32, tag="ps", name=f"ps_{h}_{kc}")
                    for kc in range(4)
                ]
                for ccp in range(NCP):
                    s = slice(2 * ccp, 2 * ccp + 2)
                    for kc in range(4):
                        nc.tensor.matmul(
                            ps[kc][:],
                            ak[:, h, s, kc * 128 : (kc + 1) * 128],
                            aq[:, h, s, :],
                            start=(ccp == 0),
                            stop=(ccp == NCP - 1),
                            perf_mode=mybir.MatmulPerfMode.DoubleRow,
                        )
                for kc in range(4):
                    ot = op.tile([128, N_CTX], bf16, tag="o", name=f"o_{h}_{kc}")
                    # odd tiles of the last head evacuate on ACT so the
                    # DVE and ACT evacuations drain the tail in parallel
                    nsub = 1
                    w = N_CTX // nsub
                    for sub in range(nsub):
                        cs = slice(sub * w, (sub + 1) * w)
                        if h == 1 and kc % 2 == 1:
                            nc.scalar.activation(
                                ot[:, cs],
                                ps[kc][:, cs],
                                mybir.ActivationFunctionType.Identity,
                                bias=biasc[:, 0:1],
                                scale=SCALE_MM,
                            )
                        else:
                            nc.vector.tensor_scalar(
                                ot[:, cs],
                                ps[kc][:, cs],
                                SCALE_MM,
                                BIAS_MM,
                                mybir.AluOpType.mult,
                                mybir.AluOpType.add,
                            )
                        oeng = nc.scalar if kc % 2 == 0 else nc.sync
                        oeng.dma_start(
                            out_d[h, kc * 128 : (kc + 1) * 128, cs], ot[:, cs]
                        )

    nc.compile()
    _CACHE["nc"] = nc
    return nc


def _encode(x):
    """x: [BS, N_CTX, N_HEADS, WIDTH] -> codes [BS, N_HEADS, 128, NCC, N_CTX] fp8."""
    import concourse.mybir as mybir

    fp8np = mybir.dt.np(mybir.dt.float8e4)
    taus = (-R + DELTA * (np.arange(T) + 0.5)).astype(np.float32)
    xt = x.transpose(0, 2, 3, 1)  # [b, h, w, j]
    bits = xt[:, :, None, :, :] > taus[None, None, :, None, None]  # [b,h,T,w,j]
    codes = np.where(bits, np.float32(0.5), np.float32(-0.5))
    # contraction row r = t*W + w; chunk cc = r // 128, partition p = r % 128
    codes = codes.reshape(BS, N_HEADS, NCC, 128, N_CTX).transpose(0, 1, 3, 2, 4)
    return np.ascontiguousarray(codes.astype(fp8np))


def kernel(q, k, _trace=False):
    from concourse.bass_utils import run_bass_kernel_spmd

    q = np.asarray(q, dtype=np.float32)
    k = np.asarray(k, dtype=np.float32)
    nc = _build()
    cq = _encode(q)  # [b, h, 128, NCC, j]
    ck = _encode(k)
    in_maps = []
    for c in range(N_CORES):
        b, hp = divmod(c, 4)
        aq = np.ascontiguousarray(
            cq[b, 2 * hp : 2 * hp + 2].transpose(1, 0, 2, 3)
        )  # [128, 2, NCC, 512]
        ak = np.ascontiguousarray(ck[b, 2 * hp : 2 * hp + 2].transpose(1, 0, 2, 3))
        in_maps.append({"aq": aq, "ak": ak})
    res = run_bass_kernel_spmd(nc, in_maps, core_ids=list(range(N_CORES)), trace=_trace)
    _CACHE["last_results"] = res
    attn = np.empty((BS, N_CTX, N_CTX, N_HEADS), np.float32)
    for c in range(N_CORES):
        b, hp = divmod(c, 4)
        o = res.results[c]["out"].astype(np.float32)
        attn[b, :, :, 2 * hp] = o[0]
        attn[b, :, :, 2 * hp + 1] = o[1]
    return attn
